# revision 36
# baseline (speedup 1.0000x reference)
"""Trainium2 Bass kernel for nn_Decoder (Hawkes intensity decoder).

Contract: kernel(**inputs) takes FULL unsharded inputs (as produced by the
reference's setup_inputs) and returns the full (lambda_src, lambda_dst,
return_time_pred) tuple.

Sharding (8 NeuronCores):
  - lambda_src/lambda_dst (B=512 x N=50000): node-sharded. Core c computes
    ALL 512 events against its 6250-node slice of all_embeddings. This cuts
    the per-core input DMA 8x vs batch-sharding (only the node slice is
    read) while output DMA (the roofline: 2 x 102.4MB fp32) is identical.
  - return_time_pred (B=512): batch-sharded, 64 events per core.

Per-core math. hawkes_intensity separates per (event b, node n):
    g[b,n] = z_ev[b].Wa[et_b] + emb[n].Wb[et_b] + bias[et_b]
             + alpha[et_b]*exp(-w_t[et_b]*td_b/100)
so with x = g/(psi+1e-7),
    lambda[b,n] = psi_e*(logaddexp(0,-x)+x) = psi_e*ln(1+exp(x)).
The node-independent part folds into a per-event scalar w_b; the node part
is a K=32 matmul row: x[b,n] = CW[b,:].emb[n,:] + w_b with
CW[b,:] = Wb[et_b]/(psi_e+1e-7). On device:
    PE   : PSUM[128ev, n] = cwT^T @ embT     (fp32r, K=32)
    ACT  : e = Exp(PSUM + w_b)  (per-partition bias)
    ACT  : l = Ln(e + 1)
    DVE  : out = l * psi_b      (per-partition scalar)
    DMA  : out -> lam[...]
|x| <~ 25 for any plausible input here, so Ln(1+Exp(x)) is overflow-safe in
fp32 and equals the reference's stable logaddexp form.

return_time_pred per core (64 events on partitions, s=0..1000 on the free
dim; tdb2 carries [s grid | trapezoid-weight*td]):
    E = Exp(s * (-w_t_b*1e-4)); F = Exp(rt_scale_b*E + rt_bias_b)
    I = psi_b*Ln(F+1)                       # intensity (b, s)
    cum = tensor_tensor_scan(I, add)        # inclusive cumsum along s (DVE)
    X = Exp(-0.01*cum); density = I*X
    rtp = reduce_sum(density * wtd, axis=s) # wtd = trapz weight * td
"""

import numpy as np

N_NODES = 50000
B = 512
D = 32
NCORES = 8
NC_NODES = N_NODES // NCORES  # 6250
NC_EV = B // NCORES  # 64
S = 1001
TRAIN_TD_MAX = 100.0
TIMESTEP = 0.01

_PROGRAM_CACHE = {}


def _build_program():
    """Build + compile the SPMD Bass program (identical on all 8 cores)."""
    import concourse.bass as bass
    import concourse.mybir as mybir
    from concourse import bacc, tile

    dt = mybir.dt
    AF = mybir.ActivationFunctionType

    # Both Exp and Ln live in the 'natural_log_exp_and_others' activation
    # table set; left to itself the table-load pass picks per-function sets
    # and the Scalar engine reloads tables on every Exp<->Ln alternation
    # (~1.3us each, ~46us total here). Restrict selection to the shared set
    # (other sets are emptied, keeping dict order so act_func_set_id indices
    # stay aligned with act_info.json).
    from concourse.hw_specs import get_activation_tables as _real_gat

    def _patched_gat(arch):
        tabs = _real_gat(arch)
        return {
            k: (v if k == "natural_log_exp_and_others" else set())
            for k, v in tabs.items()
        }

    bacc.get_activation_tables = _patched_gat

    nc = bacc.Bacc(
        "TRN2",
        target_bir_lowering=False,
        debug=False,
        num_devices=NCORES,
    )

    # ---- DRAM parameters -------------------------------------------------
    # embT/cwT are declared float32r (same 4-byte layout as the float32
    # host arrays): they are only consumed by the fp32r matmul (full-rate
    # on the PE at free>=256, vs 4 cyc/row for exact fp32; measured accuracy
    # cost is ~1e-4 scale-relative on the lambdas).
    embT_d = nc.declare_dram_parameter("embT", [D, NC_NODES], dt.float32r, isOutput=False)
    cwT_d = nc.declare_dram_parameter("cwT", [D, 1024], dt.float32r, isOutput=False)
    par_d = nc.declare_dram_parameter("par", [128, 12], dt.float32, isOutput=False)
    rtpar_d = nc.declare_dram_parameter("rtpar", [NC_EV, 4], dt.float32, isOutput=False)
    tdb_d = nc.declare_dram_parameter("tdb", [NC_EV, 2 * S], dt.float32, isOutput=False)

    lam_d = nc.declare_dram_parameter("lam", [1024, NC_NODES], dt.float32, isOutput=True)
    rtp_d = nc.declare_dram_parameter("rtp", [NC_EV, 1], dt.float32, isOutput=True)

    F32R = dt.float32r

    with tile.TileContext(nc) as tc:
        with tc.tile_pool(name="const", bufs=1) as cpool:
            # embT lives in FOUR tiles aligned with the 2048-col compute
            # chunks: Tile tracks dependencies per tile, so with a single
            # big tile the first matmul would wait for ALL embT DMAs.
            CH = 2048
            emb_chunks = []
            o = 0
            while o < NC_NODES:
                emb_chunks.append((o, min(CH, NC_NODES - o)))
                o += CH
            embTr_t = [
                cpool.tile([D, CH], F32R, name=f"embt{j}", tag=f"embt{j}")
                for j in range(len(emb_chunks))
            ]
            sb_cwTr = cpool.tile([D, 1024], F32R)
            sb_par = cpool.tile([128, 12], dt.float32)
            sb_rtpar = cpool.tile([NC_EV, 4], dt.float32)
            sb_tdb = cpool.tile([NC_EV, 2 * S], dt.float32)

            # The Activation HWDGE queue finishes its preamble ~3us before
            # SP's, so the first-matmul dependencies (cwT, embT chunk 0) and
            # the small params issue there; the remaining embT chunks stream
            # from the SP queue in parallel.
            nc.scalar.dma_start(sb_cwTr[:], cwT_d[:])
            nc.scalar.dma_start(
                embTr_t[0][:, 0 : emb_chunks[0][1]],
                embT_d[:, 0 : emb_chunks[0][1]],
            )
            nc.scalar.dma_start(sb_par[:], par_d[:])
            nc.scalar.dma_start(sb_tdb[:], tdb_d[:])
            nc.scalar.dma_start(sb_rtpar[:], rtpar_d[:])
            for j in range(1, len(emb_chunks)):
                lo, F = emb_chunks[j]
                nc.sync.dma_start(embTr_t[j][:, 0:F], embT_d[:, lo : lo + F])

            # ========== return_time_pred (64 events, no PSUM) ==========
            with tc.tile_pool(name="rt_sb", bufs=1) as rpool:
                ev = NC_EV
                E = rpool.tile([ev, S], dt.float32)
                Ften = rpool.tile([ev, S], dt.float32)
                I = rpool.tile([ev, S], dt.float32)
                cum = rpool.tile([ev, S], dt.float32)
                dens = rpool.tile([ev, S], dt.float32)
                rtp_sb = rpool.tile([ev, 1], dt.float32)

                sgrid = sb_tdb[:, 0:S]
                wtd = sb_tdb[:, S : 2 * S]

                # E = exp(-w_t/1e4 * s)
                nc.scalar.activation(E[:], sgrid, AF.Exp, scale=sb_rtpar[:, 2:3])
                # F = exp(rt_scale*E + rt_bias)
                nc.scalar.activation(
                    Ften[:], E[:], AF.Exp,
                    scale=sb_rtpar[:, 1:2], bias=sb_rtpar[:, 0:1],
                )
                # I = psi * ln(F + 1)
                nc.scalar.activation(I[:], Ften[:], AF.Ln, bias=1.0)
                nc.vector.tensor_scalar_mul(I[:], I[:], sb_rtpar[:, 3:4])

                def _emit_rtp_tail():
                    # inclusive cumsum along s (fp32 state)
                    nc.vector.tensor_tensor_scan(
                        cum[:], I[:], I[:], 0.0,
                        mybir.AluOpType.add, mybir.AluOpType.bypass,
                    )
                    # X = exp(-TIMESTEP*cum); density = I * X (reuse cum)
                    nc.scalar.activation(cum[:], cum[:], AF.Exp, scale=-TIMESTEP)
                    nc.vector.tensor_tensor(
                        dens[:], cum[:], I[:], mybir.AluOpType.mult
                    )
                    # ts = density * (trapz_w * td); rtp = sum_s ts
                    nc.vector.tensor_tensor(
                        dens[:], dens[:], wtd, mybir.AluOpType.mult
                    )
                    nc.vector.tensor_reduce(
                        rtp_sb[:], dens[:], mybir.AxisListType.X,
                        mybir.AluOpType.add,
                    )
                    nc.sync.dma_start(rtp_d[:], rtp_sb[:])

                # ========== lambda_src / lambda_dst main loop ==========
                # Row-major over the 8 (type, event-group) rows; each row
                # covers all 6250 nodes in one eb tile. Exp drains PSUM at
                # 2048 granularity; Ln runs in-place per half-row (3125) so
                # the output DMA of each half starts early and the final
                # DMA tail stays short.
                HALF = 3125
                with tc.tile_pool(name="lam_ps", bufs=2, space="PSUM") as lps, \
                     tc.tile_pool(name="lam_e", bufs=3) as epool:
                    for tg in range(8):
                        g = tg % 4
                        eb = epool.tile([128, NC_NODES], dt.float32, tag="eb")
                        # exp chunks; the very first is split at 512 so the
                        # Scalar engine starts as soon as one matmul lands
                        # (also warms the PE clock ramp on real work)
                        if tg == 0:
                            echunks = [(s, 512) for s in range(0, CH, 512)]
                            echunks += [(o, min(CH, NC_NODES - o))
                                        for o in range(CH, NC_NODES, CH)]
                        else:
                            echunks = [(o, min(CH, NC_NODES - o))
                                       for o in range(0, NC_NODES, CH)]
                        for off, F in echunks:
                            ps = lps.tile([128, CH], dt.float32, tag="lps")
                            ej = off // CH
                            el = off - ej * CH
                            for s1 in range(0, F, 512):
                                ss = min(512, F - s1)
                                nc.tensor.matmul(
                                    ps[:, s1 : s1 + ss],
                                    sb_cwTr[:, tg * 128 : (tg + 1) * 128],
                                    embTr_t[ej][:, el + s1 : el + s1 + ss],
                                    start=True,
                                    stop=True,
                                )
                            nc.scalar.activation(
                                eb[:, off : off + F], ps[:, 0:F], AF.Exp,
                                bias=sb_par[:, tg : tg + 1],
                            )
                        # Ln per half-row (few big ACT ops); psi-mult + DMA
                        # at ~quarter-row pieces so output bytes enqueue
                        # steadily; final row tapers so the kernel tail is
                        # a small transfer.
                        if tg < 7:
                            lpieces = [(0, NC_NODES)]
                        else:
                            # final row: pieced Ln, so the closing
                            # Ln->mult->DMA chain is short
                            lpieces = [(0, HALF), (HALF, 1563),
                                       (HALF + 1563, 1041),
                                       (HALF + 2604, NC_NODES - HALF - 2604)]
                        for off, F in lpieces:
                            sl = eb[:, off : off + F]
                            nc.scalar.activation(sl, sl, AF.Ln, bias=1.0)
                            mp = [(off + k, min(1563, F - k))
                                  for k in range(0, F, 1563)]
                            for moff, mF in mp:
                                msl = eb[:, moff : moff + mF]
                                nc.vector.tensor_scalar_mul(
                                    msl, msl, sb_par[:, 8 + g : 9 + g]
                                )
                                nc.sync.dma_start(
                                    lam_d[tg * 128 : (tg + 1) * 128,
                                          moff : moff + mF],
                                    msl,
                                )
                        if tg == 0:
                            # rtp integral: its DVE scan ran during row 0's
                            # ACT work; the single Exp slots in here without
                            # stalling the in-order Scalar queue.
                            _emit_rtp_tail()

    nc.compile()
    return nc


def _get_program():
    if "nc" not in _PROGRAM_CACHE:
        _PROGRAM_CACHE["nc"] = _build_program()
    return _PROGRAM_CACHE["nc"]


def _host_prep(all_embeddings, assoc, src, pos_dst, last_update, cur_time, et,
               W, b, psi, alpha, w_t):
    """Per-event scalar prep (O(B*D)) + shard layouts. float64 intermediate
    for the tiny scalar math, cast to float32 for upload."""
    emb = np.asarray(all_embeddings, dtype=np.float32)
    assoc = np.asarray(assoc).astype(np.int64)
    src = np.asarray(src).astype(np.int64)
    pos_dst = np.asarray(pos_dst).astype(np.int64)
    lu = np.asarray(last_update, dtype=np.float64)
    ct = np.asarray(cur_time, dtype=np.float64)
    e = np.asarray(et).astype(np.int64)
    e = (e > 0).astype(np.int64)
    W = np.asarray(W, dtype=np.float64)
    bb = np.asarray(b, dtype=np.float64)
    psi = np.asarray(psi, dtype=np.float64)
    alpha = np.asarray(alpha, dtype=np.float64)
    w_t = np.asarray(w_t, dtype=np.float64)

    Wu = W[:, :D]  # (2, D)
    Wv = W[:, D:]

    idx_s = assoc[src]
    idx_d = assoc[pos_dst]
    zs = emb[idx_s].astype(np.float64)  # (B, D)
    zd = emb[idx_d].astype(np.float64)
    td_s = ct - lu[idx_s]
    td_d = ct - lu[idx_d]

    invpsi = 1.0 / (psi + 1e-7)
    ip = invpsi[e]  # (B,)
    psi_e = psi[e]
    alpha_e = alpha[e]
    wt_e = w_t[e]
    b_e = bb[e]

    a_s = np.einsum("bk,bk->b", zs, Wu[e])
    a_d = np.einsum("bk,bk->b", zd, Wv[e])
    wb_s = ip * (a_s + b_e + alpha_e * np.exp(-wt_e * td_s / TRAIN_TD_MAX))
    wb_d = ip * (a_d + b_e + alpha_e * np.exp(-wt_e * td_d / TRAIN_TD_MAX))

    cw_s = ip[:, None] * Wv[e]  # (B, D)   lambda_src node side goes via Wv
    cw_d = ip[:, None] * Wu[e]  # lambda_dst node side via Wu

    # cwT (D, 1024): col block tg = t*4+g holds events g*128..(g+1)*128 of type t
    cwT = np.zeros((D, 1024), dtype=np.float32)
    par = np.zeros((128, 12), dtype=np.float32)
    for g in range(4):
        sl = slice(g * 128, (g + 1) * 128)
        cwT[:, (0 * 4 + g) * 128 : (0 * 4 + g + 1) * 128] = cw_s[sl].T
        cwT[:, (1 * 4 + g) * 128 : (1 * 4 + g + 1) * 128] = cw_d[sl].T
        par[:, 0 * 4 + g] = wb_s[sl]
        par[:, 1 * 4 + g] = wb_d[sl]
        par[:, 8 + g] = psi_e[sl]

    # rtp per-event scalars
    base = a_s + np.einsum("bk,bk->b", zd, Wv[e]) + b_e
    rt_bias = ip * base
    rt_scale = ip * alpha_e
    nws = -wt_e * (TIMESTEP / TRAIN_TD_MAX)  # exp(nws * s), s integer
    rtpar_full = np.stack(
        [rt_bias, rt_scale, nws, psi_e], axis=1
    ).astype(np.float32)  # (B, 4)

    # tdb2 (64, 2S): [s grid | trapezoid_weight * td]
    s_vals = np.arange(S, dtype=np.float64)
    w = np.full(S, TIMESTEP)
    w[-1] = 0.5 * TIMESTEP
    wtd = (w * (TIMESTEP * s_vals)).astype(np.float32)
    tdb2 = np.zeros((NC_EV, 2 * S), dtype=np.float32)
    tdb2[:, 0:S] = s_vals.astype(np.float32)[None, :]
    tdb2[:, S:] = wtd[None, :]

    # per-core embT slices (pre-transposed layout)
    embT_slices = []
    for c in range(NCORES):
        sl = emb[c * NC_NODES : (c + 1) * NC_NODES, :]
        embT_slices.append(np.ascontiguousarray(sl.T))

    rtpar_slices = [
        np.ascontiguousarray(rtpar_full[c * NC_EV : (c + 1) * NC_EV])
        for c in range(NCORES)
    ]

    return cwT, par, tdb2, embT_slices, rtpar_slices


def kernel(all_embeddings, assoc, src, pos_dst, neg_dst, last_update,
           cur_time, et, W, b, psi, alpha, w_t):
    from concourse.bass_utils import run_bass_kernel_spmd

    cwT, par, tdb2, embT_slices, rtpar_slices = _host_prep(
        all_embeddings, assoc, src, pos_dst, last_update, cur_time, et,
        W, b, psi, alpha, w_t,
    )

    nc = _get_program()

    in_maps = []
    for c in range(NCORES):
        in_maps.append({
            "embT": embT_slices[c],
            "cwT": cwT,
            "par": par,
            "rtpar": rtpar_slices[c],
            "tdb": tdb2,
        })

    res = run_bass_kernel_spmd(nc, in_maps, core_ids=list(range(NCORES))).results

    lam_parts = [res[c]["lam"] for c in range(NCORES)]  # (1024, 6250) each
    lambda_src = np.concatenate([p[:512] for p in lam_parts], axis=1)
    lambda_dst = np.concatenate([p[512:] for p in lam_parts], axis=1)
    rtp = np.concatenate([res[c]["rtp"].reshape(NC_EV) for c in range(NCORES)])
    return lambda_src, lambda_dst, rtp


# revision 37
# speedup vs baseline: 1.0277x; 1.0277x over previous
"""Trainium2 Bass kernel for nn_Decoder (Hawkes intensity decoder).

Contract: kernel(**inputs) takes FULL unsharded inputs (as produced by the
reference's setup_inputs) and returns the full (lambda_src, lambda_dst,
return_time_pred) tuple.

Sharding (8 NeuronCores):
  - lambda_src/lambda_dst (B=512 x N=50000): node-sharded. Core c computes
    ALL 512 events against its 6250-node slice of all_embeddings. This cuts
    the per-core input DMA 8x vs batch-sharding (only the node slice is
    read) while output DMA (the roofline: 2 x 102.4MB fp32) is identical.
  - return_time_pred (B=512): batch-sharded, 64 events per core.

Per-core math. hawkes_intensity separates per (event b, node n):
    g[b,n] = z_ev[b].Wa[et_b] + emb[n].Wb[et_b] + bias[et_b]
             + alpha[et_b]*exp(-w_t[et_b]*td_b/100)
so with x = g/(psi+1e-7),
    lambda[b,n] = psi_e*(logaddexp(0,-x)+x) = psi_e*ln(1+exp(x)).
The node-independent part folds into a per-event scalar w_b; the node part
is a K=32 matmul row: x[b,n] = CW[b,:].emb[n,:] + w_b with
CW[b,:] = Wb[et_b]/(psi_e+1e-7). On device:
    PE   : PSUM[128ev, n] = cwT^T @ embT     (fp32r, K=32)
    ACT  : e = Exp(PSUM + w_b)  (per-partition bias)
    ACT  : l = Ln(e + 1)
    DVE  : out = l * psi_b      (per-partition scalar)
    DMA  : out -> lam[...]
|x| <~ 25 for any plausible input here, so Ln(1+Exp(x)) is overflow-safe in
fp32 and equals the reference's stable logaddexp form.

return_time_pred per core (64 events on partitions, s=0..1000 on the free
dim; tdb2 carries [s grid | trapezoid-weight*td]):
    E = Exp(s * (-w_t_b*1e-4)); F = Exp(rt_scale_b*E + rt_bias_b)
    I = psi_b*Ln(F+1)                       # intensity (b, s)
    cum = tensor_tensor_scan(I, add)        # inclusive cumsum along s (DVE)
    X = Exp(-0.01*cum); density = I*X
    rtp = reduce_sum(density * wtd, axis=s) # wtd = trapz weight * td
"""

import numpy as np

N_NODES = 50000
B = 512
D = 32
NCORES = 8
NC_NODES = N_NODES // NCORES  # 6250
NC_EV = B // NCORES  # 64
S = 1001
TRAIN_TD_MAX = 100.0
TIMESTEP = 0.01

_PROGRAM_CACHE = {}


def _build_program():
    """Build + compile the SPMD Bass program (identical on all 8 cores)."""
    import concourse.bass as bass
    import concourse.mybir as mybir
    from concourse import bacc, tile

    dt = mybir.dt
    AF = mybir.ActivationFunctionType

    # Both Exp and Ln live in the 'natural_log_exp_and_others' activation
    # table set; left to itself the table-load pass picks per-function sets
    # and the Scalar engine reloads tables on every Exp<->Ln alternation
    # (~1.3us each, ~46us total here). Restrict selection to the shared set
    # (other sets are emptied, keeping dict order so act_func_set_id indices
    # stay aligned with act_info.json).
    from concourse.hw_specs import get_activation_tables as _real_gat

    def _patched_gat(arch):
        tabs = _real_gat(arch)
        return {
            k: (v if k == "natural_log_exp_and_others" else set())
            for k, v in tabs.items()
        }

    bacc.get_activation_tables = _patched_gat

    nc = bacc.Bacc(
        "TRN2",
        target_bir_lowering=False,
        debug=False,
        num_devices=NCORES,
    )

    # ---- DRAM parameters -------------------------------------------------
    # embT/cwT are declared float32r (same 4-byte layout as the float32
    # host arrays): they are only consumed by the fp32r matmul (full-rate
    # on the PE at free>=256, vs 4 cyc/row for exact fp32; measured accuracy
    # cost is ~1e-4 scale-relative on the lambdas).
    embT_d = nc.declare_dram_parameter("embT", [D, NC_NODES], dt.float32r, isOutput=False)
    cwT_d = nc.declare_dram_parameter("cwT", [D, 1024], dt.float32r, isOutput=False)
    par_d = nc.declare_dram_parameter("par", [128, 12], dt.float32, isOutput=False)
    rtpar_d = nc.declare_dram_parameter("rtpar", [NC_EV, 4], dt.float32, isOutput=False)
    tdb_d = nc.declare_dram_parameter("tdb", [NC_EV, 2 * S], dt.float32, isOutput=False)

    lam_d = nc.declare_dram_parameter("lam", [1024, NC_NODES], dt.float32, isOutput=True)
    rtp_d = nc.declare_dram_parameter("rtp", [NC_EV, 1], dt.float32, isOutput=True)

    F32R = dt.float32r

    with tile.TileContext(nc) as tc:
        with tc.tile_pool(name="const", bufs=1) as cpool:
            # embT lives in FOUR tiles aligned with the 2048-col compute
            # chunks: Tile tracks dependencies per tile, so with a single
            # big tile the first matmul would wait for ALL embT DMAs.
            CH = 2048
            emb_chunks = []
            o = 0
            while o < NC_NODES:
                emb_chunks.append((o, min(CH, NC_NODES - o)))
                o += CH
            embTr_t = [
                cpool.tile([D, CH], F32R, name=f"embt{j}", tag=f"embt{j}")
                for j in range(len(emb_chunks))
            ]
            sb_cwTr = cpool.tile([D, 1024], F32R)
            sb_par = cpool.tile([128, 12], dt.float32)
            sb_rtpar = cpool.tile([NC_EV, 4], dt.float32)
            sb_tdb = cpool.tile([NC_EV, 2 * S], dt.float32)

            # The Activation HWDGE queue finishes its preamble ~3us before
            # SP's, so the first-matmul dependencies (cwT, embT chunk 0) and
            # the small params issue there; the remaining embT chunks stream
            # from the SP queue in parallel.
            nc.scalar.dma_start(sb_cwTr[:], cwT_d[:])
            nc.scalar.dma_start(
                embTr_t[0][:, 0 : emb_chunks[0][1]],
                embT_d[:, 0 : emb_chunks[0][1]],
            )
            nc.scalar.dma_start(sb_par[:], par_d[:])
            nc.scalar.dma_start(sb_tdb[:], tdb_d[:])
            nc.scalar.dma_start(sb_rtpar[:], rtpar_d[:])
            for j in range(1, len(emb_chunks)):
                lo, F = emb_chunks[j]
                nc.sync.dma_start(embTr_t[j][:, 0:F], embT_d[:, lo : lo + F])

            # ========== return_time_pred (64 events, no PSUM) ==========
            with tc.tile_pool(name="rt_sb", bufs=1) as rpool:
                ev = NC_EV
                E = rpool.tile([ev, S], dt.float32)
                Ften = rpool.tile([ev, S], dt.float32)
                I = rpool.tile([ev, S], dt.float32)
                cum = rpool.tile([ev, S], dt.float32)
                dens = rpool.tile([ev, S], dt.float32)
                rtp_sb = rpool.tile([ev, 1], dt.float32)

                sgrid = sb_tdb[:, 0:S]
                wtd = sb_tdb[:, S : 2 * S]

                # E = exp(-w_t/1e4 * s)
                nc.scalar.activation(E[:], sgrid, AF.Exp, scale=sb_rtpar[:, 2:3])
                # F = exp(rt_scale*E + rt_bias)
                nc.scalar.activation(
                    Ften[:], E[:], AF.Exp,
                    scale=sb_rtpar[:, 1:2], bias=sb_rtpar[:, 0:1],
                )
                # I = psi * ln(F + 1)
                nc.scalar.activation(I[:], Ften[:], AF.Ln, bias=1.0)
                nc.vector.tensor_scalar_mul(I[:], I[:], sb_rtpar[:, 3:4])

                def _emit_rtp_tail():
                    # inclusive cumsum along s (fp32 state)
                    nc.vector.tensor_tensor_scan(
                        cum[:], I[:], I[:], 0.0,
                        mybir.AluOpType.add, mybir.AluOpType.bypass,
                    )
                    # X = exp(-TIMESTEP*cum); density = I * X (reuse cum)
                    nc.scalar.activation(cum[:], cum[:], AF.Exp, scale=-TIMESTEP)
                    nc.vector.tensor_tensor(
                        dens[:], cum[:], I[:], mybir.AluOpType.mult
                    )
                    # ts = density * (trapz_w * td); rtp = sum_s ts
                    nc.vector.tensor_tensor(
                        dens[:], dens[:], wtd, mybir.AluOpType.mult
                    )
                    nc.vector.tensor_reduce(
                        rtp_sb[:], dens[:], mybir.AxisListType.X,
                        mybir.AluOpType.add,
                    )
                    nc.sync.dma_start(rtp_d[:], rtp_sb[:])

                # ========== lambda_src / lambda_dst main loop ==========
                # Row-major over the 8 (type, event-group) rows; each row
                # covers all 6250 nodes in one eb tile. Exp drains PSUM at
                # 2048 granularity; Ln runs in-place per half-row (3125) so
                # the output DMA of each half starts early and the final
                # DMA tail stays short.
                HALF = 3125
                with tc.tile_pool(name="lam_ps", bufs=2, space="PSUM") as lps, \
                     tc.tile_pool(name="lam_e", bufs=3) as epool:
                    for tg in range(8):
                        g = tg % 4
                        eb = epool.tile([128, NC_NODES], dt.float32, tag="eb")
                        # exp chunks; the very first is split at 512 so the
                        # Scalar engine starts as soon as one matmul lands
                        # (also warms the PE clock ramp on real work)
                        if tg == 0:
                            echunks = [(s, 512) for s in range(0, CH, 512)]
                            echunks += [(o, min(CH, NC_NODES - o))
                                        for o in range(CH, NC_NODES, CH)]
                        else:
                            echunks = [(o, min(CH, NC_NODES - o))
                                       for o in range(0, NC_NODES, CH)]
                        for off, F in echunks:
                            ps = lps.tile([128, CH], dt.float32, tag="lps")
                            ej = off // CH
                            el = off - ej * CH
                            for s1 in range(0, F, 512):
                                ss = min(512, F - s1)
                                nc.tensor.matmul(
                                    ps[:, s1 : s1 + ss],
                                    sb_cwTr[:, tg * 128 : (tg + 1) * 128],
                                    embTr_t[ej][:, el + s1 : el + s1 + ss],
                                    start=True,
                                    stop=True,
                                )
                            nc.scalar.activation(
                                eb[:, off : off + F], ps[:, 0:F], AF.Exp,
                                bias=sb_par[:, tg : tg + 1],
                            )
                        # Ln per half-row (few big ACT ops); psi-mult + DMA
                        # at ~quarter-row pieces so output bytes enqueue
                        # steadily; final row tapers so the kernel tail is
                        # a small transfer.
                        if tg < 7:
                            lpieces = [(0, HALF), (HALF, NC_NODES - HALF)]
                        else:
                            # final row: pieced Ln, so the closing
                            # Ln->mult->DMA chain is short
                            lpieces = [(0, HALF), (HALF, 1563),
                                       (HALF + 1563, 1041),
                                       (HALF + 2604, NC_NODES - HALF - 2604)]
                        for off, F in lpieces:
                            sl = eb[:, off : off + F]
                            nc.scalar.activation(sl, sl, AF.Ln, bias=1.0)
                            mp = [(off + k, min(1563, F - k))
                                  for k in range(0, F, 1563)]
                            for moff, mF in mp:
                                msl = eb[:, moff : moff + mF]
                                nc.vector.tensor_scalar_mul(
                                    msl, msl, sb_par[:, 8 + g : 9 + g]
                                )
                                nc.sync.dma_start(
                                    lam_d[tg * 128 : (tg + 1) * 128,
                                          moff : moff + mF],
                                    msl,
                                )
                        if tg == 0:
                            # rtp integral: its DVE scan ran during row 0's
                            # ACT work; the single Exp slots in here without
                            # stalling the in-order Scalar queue.
                            _emit_rtp_tail()

    nc.compile()
    return nc


def _get_program():
    if "nc" not in _PROGRAM_CACHE:
        _PROGRAM_CACHE["nc"] = _build_program()
    return _PROGRAM_CACHE["nc"]


def _host_prep(all_embeddings, assoc, src, pos_dst, last_update, cur_time, et,
               W, b, psi, alpha, w_t):
    """Per-event scalar prep (O(B*D)) + shard layouts. float64 intermediate
    for the tiny scalar math, cast to float32 for upload."""
    emb = np.asarray(all_embeddings, dtype=np.float32)
    assoc = np.asarray(assoc).astype(np.int64)
    src = np.asarray(src).astype(np.int64)
    pos_dst = np.asarray(pos_dst).astype(np.int64)
    lu = np.asarray(last_update, dtype=np.float64)
    ct = np.asarray(cur_time, dtype=np.float64)
    e = np.asarray(et).astype(np.int64)
    e = (e > 0).astype(np.int64)
    W = np.asarray(W, dtype=np.float64)
    bb = np.asarray(b, dtype=np.float64)
    psi = np.asarray(psi, dtype=np.float64)
    alpha = np.asarray(alpha, dtype=np.float64)
    w_t = np.asarray(w_t, dtype=np.float64)

    Wu = W[:, :D]  # (2, D)
    Wv = W[:, D:]

    idx_s = assoc[src]
    idx_d = assoc[pos_dst]
    zs = emb[idx_s].astype(np.float64)  # (B, D)
    zd = emb[idx_d].astype(np.float64)
    td_s = ct - lu[idx_s]
    td_d = ct - lu[idx_d]

    invpsi = 1.0 / (psi + 1e-7)
    ip = invpsi[e]  # (B,)
    psi_e = psi[e]
    alpha_e = alpha[e]
    wt_e = w_t[e]
    b_e = bb[e]

    a_s = np.einsum("bk,bk->b", zs, Wu[e])
    a_d = np.einsum("bk,bk->b", zd, Wv[e])
    wb_s = ip * (a_s + b_e + alpha_e * np.exp(-wt_e * td_s / TRAIN_TD_MAX))
    wb_d = ip * (a_d + b_e + alpha_e * np.exp(-wt_e * td_d / TRAIN_TD_MAX))

    cw_s = ip[:, None] * Wv[e]  # (B, D)   lambda_src node side goes via Wv
    cw_d = ip[:, None] * Wu[e]  # lambda_dst node side via Wu

    # cwT (D, 1024): col block tg = t*4+g holds events g*128..(g+1)*128 of type t
    cwT = np.zeros((D, 1024), dtype=np.float32)
    par = np.zeros((128, 12), dtype=np.float32)
    for g in range(4):
        sl = slice(g * 128, (g + 1) * 128)
        cwT[:, (0 * 4 + g) * 128 : (0 * 4 + g + 1) * 128] = cw_s[sl].T
        cwT[:, (1 * 4 + g) * 128 : (1 * 4 + g + 1) * 128] = cw_d[sl].T
        par[:, 0 * 4 + g] = wb_s[sl]
        par[:, 1 * 4 + g] = wb_d[sl]
        par[:, 8 + g] = psi_e[sl]

    # rtp per-event scalars
    base = a_s + np.einsum("bk,bk->b", zd, Wv[e]) + b_e
    rt_bias = ip * base
    rt_scale = ip * alpha_e
    nws = -wt_e * (TIMESTEP / TRAIN_TD_MAX)  # exp(nws * s), s integer
    rtpar_full = np.stack(
        [rt_bias, rt_scale, nws, psi_e], axis=1
    ).astype(np.float32)  # (B, 4)

    # tdb2 (64, 2S): [s grid | trapezoid_weight * td]
    s_vals = np.arange(S, dtype=np.float64)
    w = np.full(S, TIMESTEP)
    w[-1] = 0.5 * TIMESTEP
    wtd = (w * (TIMESTEP * s_vals)).astype(np.float32)
    tdb2 = np.zeros((NC_EV, 2 * S), dtype=np.float32)
    tdb2[:, 0:S] = s_vals.astype(np.float32)[None, :]
    tdb2[:, S:] = wtd[None, :]

    # per-core embT slices (pre-transposed layout)
    embT_slices = []
    for c in range(NCORES):
        sl = emb[c * NC_NODES : (c + 1) * NC_NODES, :]
        embT_slices.append(np.ascontiguousarray(sl.T))

    rtpar_slices = [
        np.ascontiguousarray(rtpar_full[c * NC_EV : (c + 1) * NC_EV])
        for c in range(NCORES)
    ]

    return cwT, par, tdb2, embT_slices, rtpar_slices


def kernel(all_embeddings, assoc, src, pos_dst, neg_dst, last_update,
           cur_time, et, W, b, psi, alpha, w_t):
    from concourse.bass_utils import run_bass_kernel_spmd

    cwT, par, tdb2, embT_slices, rtpar_slices = _host_prep(
        all_embeddings, assoc, src, pos_dst, last_update, cur_time, et,
        W, b, psi, alpha, w_t,
    )

    nc = _get_program()

    in_maps = []
    for c in range(NCORES):
        in_maps.append({
            "embT": embT_slices[c],
            "cwT": cwT,
            "par": par,
            "rtpar": rtpar_slices[c],
            "tdb": tdb2,
        })

    res = run_bass_kernel_spmd(nc, in_maps, core_ids=list(range(NCORES))).results

    lam_parts = [res[c]["lam"] for c in range(NCORES)]  # (1024, 6250) each
    lambda_src = np.concatenate([p[:512] for p in lam_parts], axis=1)
    lambda_dst = np.concatenate([p[512:] for p in lam_parts], axis=1)
    rtp = np.concatenate([res[c]["rtp"].reshape(NC_EV) for c in range(NCORES)])
    return lambda_src, lambda_dst, rtp
